# revision 1
# baseline (speedup 1.0000x reference)
"""Trainium2 Bass kernel for ContextQuestionAttention (BiDAF-style).

Reference computation (per example):
    w1, w2, w3 = w[:H], w[H:2H], w[2H:]
    S[i,j] = C[i]·w1 + Q[j]·w2 + sum_h C[i,h] Q[j,h] w3[h]
    S = where(q_mask==0, -1e9, S)
    A = softmax_j(S) @ Q
    B_att = softmax_i(max_j S); B_vec = B_att @ C
    out = concat([C, A, C*A, C*B_vec], -1)

Sharding: data-parallel over batch, 4 examples per core on 8 cores.

Kernel strategy (per example):
  - V^T[j,i] = s_cq^T + s_q[j] + maskbias[j] computed with j on partitions:
    matmul(lhsT=(w3*Q)^T chunks, rhs=C^T chunks) accumulated in PSUM; the
    per-partition (s_q + mask) bias is folded into the Exp activation that
    produces P'T = exp(V^T) in SBUF.  The s_c[i] term is constant along the
    softmax axis j and cancels in softmax_j, so it is left out here.
  - A[i,:] = P'T[:,i].T @ Q / Z'[i] — P'T is already the lhsT the A-matmul
    needs (no P transpose); Z' via an extra ones-column matmul.
  - maxS[i] = log(max_j P'[j,i]) + s_c[i]  =>  E = exp(maxS) =
    rowmax_j(P' natural) * exp(s_c).  P' natural tiles come from PE
    transposes of P'T; s_c columns via PE matmuls against C^T (reuses CT).
  - B_vec^T chunks via N=1 matmuls contracting i (lhsT = C tiles),
    transposed to a row and broadcast across partitions with a K=1 matmul.
  - exp() without max subtraction is safe: |S| <~ 13 for these magnitudes.
  - Outputs staged in SBUF as [C|A|C*A] so each i-tile needs just two
    output DMAs (6KB-row + 2KB-row bursts).
"""

import os
import sys
from contextlib import ExitStack

import numpy as np

for _p in ("/opt/trn_rl_repo", "/root/.axon_site/_ro/trn_rl_repo"):
    if os.path.isdir(_p) and _p not in sys.path:
        sys.path.append(_p)

import concourse.bass as bass
import concourse.tile as tile
from concourse import bacc, mybir
from concourse.bass_utils import run_bass_kernel_spmd

F32 = mybir.dt.float32
I32 = mybir.dt.int32
AX = mybir.AxisListType
ALU = mybir.AluOpType
ACTF = mybir.ActivationFunctionType
ts = bass.ts

N_CORES = 8
B_TOTAL = 32
B_PER_CORE = B_TOTAL // N_CORES  # 4
CLEN = 1024
QLEN = 128
H = 512
NT = CLEN // 128  # 8 i-tiles per example
KH = H // 128     # 4 h-chunks
NEG = -1.0e9


def _emit_prep(nc, pools, aps, b):
    """Everything up to E: loads, C^T, s_c, S^T->P'T, rowmax, E.
    Returns the state out-phases need."""
    (c_pool, ct_pool, q_pool, pt_pool, sm_pool, scr_pool, a_pool, ot_pool,
     p_mm, p_sm, p_ty) = pools
    C_ap, Q_ap, M_ap, O_ap, consts = aps
    ident, ones_row, ones_col, w3c, w1c, W2b = consts

    # ---- loads: C in two 4.2MB DMAs (few issues, 2KB bursts) ----
    call = c_pool.tile([128, NT * H], F32, tag="call", bufs=2)
    chalf = C_ap[b].rearrange("(g t p) h -> g p t h", g=2, p=128)
    for g in range(2):
        nc.sync.dma_start(call[:, ts(g, NT * H // 2)], chalf[g])
    Csb = [call[:, ts(t, H)] for t in range(NT)]
    Qsb = q_pool.tile([128, H], F32, tag="q", bufs=2)
    nc.sync.dma_start(Qsb[:], Q_ap[b])
    msk = sm_pool.tile([128, 1], I32, tag="msk", bufs=2)
    nc.sync.dma_start(msk[:], M_ap[b].rearrange("(p a) -> p a", a=1))

    # ---- mask bias + s_q (per-partition over j) ----
    mskf = sm_pool.tile([128, 1], F32, tag="mskf", bufs=2)
    nc.vector.tensor_copy(mskf[:], msk[:])
    mb = sm_pool.tile([128, 1], F32, tag="mb", bufs=2)
    # (mask - 1) * 1e9  -> 0 where mask==1, -1e9 where mask==0
    nc.vector.tensor_scalar(
        out=mb[:], in0=mskf[:], scalar1=1.0, scalar2=1.0e9,
        op0=ALU.subtract, op1=ALU.mult)
    scr = scr_pool.tile([128, H], F32, tag="scr", bufs=2)
    sq = sm_pool.tile([128, 1], F32, tag="sq", bufs=2)
    sqe = sm_pool.tile([128, 1], F32, tag="sqe", bufs=2)
    # sqe[j] = mb[j] + sum_h Q[j,h] * w2[h]
    nc.vector.tensor_mul(scr[:], Qsb[:], W2b[:])
    nc.vector.reduce_sum(sq[:], scr[:], axis=AX.X)
    nc.vector.tensor_add(sqe[:], sq[:], mb[:])

    # ---- (w3 * Q)^T chunks ----
    QW3T = q_pool.tile([128, H], F32, tag="qw3t", bufs=2)
    for k in range(KH):
        pqt = p_sm.tile([128, 128], F32, tag="sm", bufs=2)
        nc.tensor.transpose(pqt[:], Qsb[:, ts(k, 128)], ident[:])
        nc.vector.tensor_scalar_mul(QW3T[:, ts(k, 128)], pqt[:], w3c[k][:])

    # ---- C^T chunks (PE transposes, batched 4-per-psum-bank) ----
    CT = [ct_pool.tile([128, CLEN], F32, tag=f"ct{k}", bufs=2,
                       name=f"ct{k}_{b}")
          for k in range(KH)]
    for half in range(2):
        for k in range(KH):
            pct = p_mm.tile([128, 512], F32, tag="mm", bufs=4)
            for tt in range(4):
                t = half * 4 + tt
                nc.tensor.transpose(
                    pct[:, ts(tt, 128)], Csb[t][:, ts(k, 128)], ident[:])
            if k % 2 == 0:
                nc.scalar.copy(CT[k][:, ts(half, 512)], pct[:])
            else:
                nc.vector.tensor_copy(CT[k][:, ts(half, 512)], pct[:])

    # ---- s_c columns on PE (reuses CT): SC[:, t] = C_t @ w1 ----
    # 8 sequential accumulation groups share one PSUM bank (one per column)
    SC = sm_pool.tile([128, NT], F32, tag="sc", bufs=2)
    psc8 = p_ty.tile([128, NT], F32, tag="tiny", bufs=2)
    for t in range(NT):
        for k in range(KH):
            nc.tensor.matmul(psc8[:, t:t + 1], CT[k][:, ts(t, 128)],
                             w1c[k][:], start=(k == 0), stop=(k == KH - 1))
    nc.scalar.copy(SC[:], psc8[:])

    # ---- S^T matmul + fused bias/exp -> P'T ----
    PT = pt_pool.tile([128, CLEN], F32, tag="pt", bufs=2)
    for n in range(2):
        pst = p_mm.tile([128, 512], F32, tag="mm", bufs=4)
        for k in range(KH):
            nc.tensor.matmul(
                pst[:], QW3T[:, ts(k, 128)], CT[k][:, ts(n, 512)],
                start=(k == 0), stop=(k == KH - 1))
        # P'T = exp(s_cq^T + s_q + maskbias)
        nc.scalar.activation(PT[:, ts(n, 512)], pst[:], ACTF.Exp,
                             bias=sqe[:], scale=1.0)

    # ---- row max of P' natural (via PE transpose back) ----
    MXE = sm_pool.tile([128, NT], F32, tag="mxe", bufs=2)
    for t in range(NT):
        ppn = p_sm.tile([128, 128], F32, tag="sm", bufs=2)
        nc.tensor.transpose(ppn[:], PT[:, ts(t, 128)], ident[:])
        nc.vector.reduce_max(MXE[:, t:t + 1], ppn[:], axis=AX.X)

    # ---- E = exp(maxS) = rowmax(P') * exp(s_c) ----
    esc = sm_pool.tile([128, NT], F32, tag="esc", bufs=2)
    nc.scalar.activation(esc[:], SC[:], ACTF.Exp)
    E = sm_pool.tile([128, NT], F32, tag="e", bufs=2)
    nc.vector.tensor_mul(E[:], MXE[:], esc[:])

    return dict(b=b, call=call, Csb=Csb, Qsb=Qsb, PT=PT, E=E)


def _emit_outA(nc, pools, aps, st):
    (c_pool, ct_pool, q_pool, pt_pool, sm_pool, scr_pool, a_pool, ot_pool,
     p_mm, p_sm, p_ty) = pools
    C_ap, Q_ap, M_ap, O_ap, consts = aps
    ident, ones_row, ones_col, w3c, w1c, W2b = consts
    b, Csb, Qsb, PT = st["b"], st["Csb"], st["Qsb"], st["PT"]

    if True:
        # ---- A path per i-tile; stage [C|A|C*A] and DMA as one 6KB-row burst --
        # Z' for 4 tiles batched per PSUM bank; one reciprocal per batch.
        RZP = sm_pool.tile([128, NT], F32, tag="rzp", bufs=2)
        for g in range(2):
            pzg = p_ty.tile([128, 4], F32, tag="tiny", bufs=2)
            for tt in range(4):
                t = g * 4 + tt
                nc.tensor.matmul(pzg[:, tt:tt + 1], PT[:, ts(t, 128)], ones_col,
                                 start=True, stop=True)
            nc.vector.reciprocal(RZP[:, ts(g, 4)], pzg[:])
        for t in range(NT):
            pa = p_mm.tile([128, 512], F32, tag="mm", bufs=4)
            nc.tensor.matmul(pa[:], PT[:, ts(t, 128)], Qsb[:],
                             start=True, stop=True)
            ot = ot_pool.tile([128, 3 * H], F32, tag="ot", bufs=3)
            nc.gpsimd.tensor_copy(ot[:, 0:H], Csb[t][:])
            nc.scalar.mul(ot[:, H:2 * H], pa[:], RZP[:, t:t + 1])
            nc.vector.tensor_mul(ot[:, 2 * H:3 * H], Csb[t][:], ot[:, H:2 * H])
            nc.sync.dma_start(O_ap[b, ts(t, 128), 0:3 * H], ot[:])


def _emit_outB(nc, pools, aps, st):
    (c_pool, ct_pool, q_pool, pt_pool, sm_pool, scr_pool, a_pool, ot_pool,
     p_mm, p_sm, p_ty) = pools
    C_ap, Q_ap, M_ap, O_ap, consts = aps
    ident, ones_row, ones_col, w3c, w1c, W2b = consts
    b, Csb, E = st["b"], st["Csb"], st["E"]

    if True:
        # ---- B path: B_vec^T chunks via N=1 matmuls contracting i ----
        # 4 sequential accumulation groups (one per chunk column) in one bank
        pbt4 = p_ty.tile([128, KH], F32, tag="tiny", bufs=2)
        for k in range(KH):
            for t in range(NT):
                nc.tensor.matmul(pbt4[:, k:k + 1], Csb[t][:, ts(k, 128)],
                                 E[:, t:t + 1], start=(t == 0),
                                 stop=(t == NT - 1))
        btc = sm_pool.tile([128, KH], F32, tag="btc", bufs=2)
        nc.scalar.copy(btc[:], pbt4[:])
        Btrow = sm_pool.tile([1, H], F32, tag="btrow", bufs=2)
        for k in range(KH):
            ptr = p_sm.tile([1, 128], F32, tag="sm", bufs=2)
            nc.tensor.transpose(ptr[:], btc[:, k:k + 1], ident[:])
            nc.scalar.copy(Btrow[:, ts(k, 128)], ptr[:])
        # Z2 = sum(E): free-dim reduce on DVE, partition reduce via one matmul
        rse = sm_pool.tile([128, 1], F32, tag="rse", bufs=2)
        nc.vector.reduce_sum(rse[:], E[:], axis=AX.X)
        pz2 = p_ty.tile([1, 1], F32, tag="tiny", bufs=2)
        nc.tensor.matmul(pz2[:], rse[:], ones_col, start=True, stop=True)
        z2sb = sm_pool.tile([1, 1], F32, tag="z2", bufs=2)
        nc.scalar.copy(z2sb[:], pz2[:])
        # broadcast row -> all partitions with K=1 matmuls
        pbb = p_mm.tile([128, 512], F32, tag="mm", bufs=4)
        nc.tensor.matmul(pbb[:], ones_row[:], Btrow[:], start=True, stop=True)
        pzb = p_ty.tile([128, 1], F32, tag="tiny", bufs=2)
        nc.tensor.matmul(pzb[:], ones_row[:], z2sb[:], start=True, stop=True)
        rzb = sm_pool.tile([128, 1], F32, tag="rzb", bufs=2)
        nc.vector.reciprocal(rzb[:], pzb[:])
        Bb = a_pool.tile([128, H], F32, tag="bb", bufs=2)
        nc.scalar.mul(Bb[:], pbb[:], rzb[:])
        # all 8 C*B_vec tiles staged in one SBUF tile -> one 8.4MB DMA
        cball = a_pool.tile([128, NT * H], F32, tag="cball", bufs=2)
        for t in range(NT):
            nc.vector.tensor_mul(cball[:, ts(t, H)], Csb[t][:], Bb[:])
        nc.sync.dma_start(
            O_ap[b][:, 3 * H:4 * H].rearrange("(t p) h -> p t h", p=128),
            cball[:])


def build_nc(n_rep: int = 1):
    nc = bacc.Bacc("TRN2", target_bir_lowering=False, debug=False,
                   num_devices=N_CORES)
    C_ap = nc.dram_tensor("C", [B_PER_CORE, CLEN, H], F32,
                          kind="ExternalInput").ap()
    Q_ap = nc.dram_tensor("Q", [B_PER_CORE, QLEN, H], F32,
                          kind="ExternalInput").ap()
    M_ap = nc.dram_tensor("q_mask", [B_PER_CORE, QLEN], I32,
                          kind="ExternalInput").ap()
    W_ap = nc.dram_tensor("w", [3 * H], F32, kind="ExternalInput").ap()
    ID_ap = nc.dram_tensor("ident", [128, 128], F32,
                           kind="ExternalInput").ap()
    O_ap = nc.dram_tensor("out", [B_PER_CORE, CLEN, 4 * H], F32,
                          kind="ExternalOutput").ap()

    with tile.TileContext(nc) as tc, ExitStack() as ctx:
        const_pool = ctx.enter_context(tc.tile_pool(name="const", bufs=1))
        c_pool = ctx.enter_context(tc.tile_pool(name="cpool", bufs=2))
        ct_pool = ctx.enter_context(tc.tile_pool(name="ctpool", bufs=2))
        q_pool = ctx.enter_context(tc.tile_pool(name="qpool", bufs=2))
        pt_pool = ctx.enter_context(tc.tile_pool(name="ptpool", bufs=2))
        sm_pool = ctx.enter_context(tc.tile_pool(name="smpool", bufs=2))
        scr_pool = ctx.enter_context(tc.tile_pool(name="scrpool", bufs=2))
        a_pool = ctx.enter_context(tc.tile_pool(name="apool", bufs=3))
        ot_pool = ctx.enter_context(tc.tile_pool(name="otpool", bufs=3))
        p_mm = ctx.enter_context(tc.tile_pool(name="pmm", bufs=4,
                                              space="PSUM"))
        p_sm = ctx.enter_context(tc.tile_pool(name="psm", bufs=2,
                                              space="PSUM"))
        p_ty = ctx.enter_context(tc.tile_pool(name="pty", bufs=2,
                                              space="PSUM"))

        # constants
        ident = const_pool.tile([128, 128], F32, tag="ident")
        nc.sync.dma_start(ident[:], ID_ap[:])
        ones_row = const_pool.tile([1, 128], F32, tag="ones_row")
        nc.vector.memset(ones_row[:], 1.0)
        ones_col = nc.const_aps.tensor(1.0, (128, 1))
        wsb = const_pool.tile([128, 12], F32, tag="wsb")
        nc.sync.dma_start(wsb[:], W_ap.rearrange("(c p) -> p c", p=128))
        w1c = [wsb[:, k:k + 1] for k in range(KH)]
        w3c = [wsb[:, 8 + k:9 + k] for k in range(KH)]
        w2r = const_pool.tile([1, H], F32, tag="w2r")
        nc.sync.dma_start(w2r[:], W_ap[H:2 * H].rearrange("(a h) -> a h", a=1))
        # broadcast w2 across partitions via K=1 matmul
        W2b = const_pool.tile([128, H], F32, tag="w2b")
        pw = p_mm.tile([128, 512], F32, tag="mm", bufs=4)
        nc.tensor.matmul(pw[:], ones_row[:], w2r[:], start=True, stop=True)
        nc.vector.tensor_copy(W2b[:], pw[:])

        consts = (ident, ones_row, ones_col, w3c, w1c, W2b)
        pools = (c_pool, ct_pool, q_pool, pt_pool, sm_pool, scr_pool, a_pool,
                 ot_pool, p_mm, p_sm, p_ty)
        aps = (C_ap, Q_ap, M_ap, O_ap, consts)

        for _rep in range(n_rep):
            for b in range(B_PER_CORE):
                st = _emit_prep(nc, pools, aps, b)
                _emit_outA(nc, pools, aps, st)
                _emit_outB(nc, pools, aps, st)

    nc.compile()
    return nc


_NC_CACHE: dict = {}


def _get_nc(n_rep: int = 1):
    key = ("nc", n_rep)
    if key not in _NC_CACHE:
        _NC_CACHE[key] = build_nc(n_rep)
    return _NC_CACHE[key]


def make_in_maps(C, Q, q_mask, w):
    ident = np.eye(128, dtype=np.float32)
    w = np.ascontiguousarray(w, dtype=np.float32)
    in_maps = []
    for c in range(N_CORES):
        sl = slice(c * B_PER_CORE, (c + 1) * B_PER_CORE)
        in_maps.append({
            "C": np.ascontiguousarray(C[sl], dtype=np.float32),
            "Q": np.ascontiguousarray(Q[sl], dtype=np.float32),
            "q_mask": np.ascontiguousarray(q_mask[sl], dtype=np.int32),
            "w": w,
            "ident": ident,
        })
    return in_maps


def kernel(C, Q, q_mask, w):
    nc = _get_nc(1)
    in_maps = make_in_maps(C, Q, q_mask, w)
    res = run_bass_kernel_spmd(nc, in_maps, list(range(N_CORES)))
    out = np.concatenate([res.results[c]["out"] for c in range(N_CORES)],
                         axis=0)
    return out

